# revision 26
# baseline (speedup 1.0000x reference)
"""Cross-attention Trainium2 Bass kernel.

Sharding: data-parallel over batch — 16 batches across 8 cores, 2 per core.
Weights replicated. Each core computes its 2 batches fully; no collectives.

x and the weights are converted to bf16 on the host (input prep, like the
sharding itself) and DMA'd directly; all matmuls run in bf16 (1 cycle/row on
TRN2 PE, fp32 PSUM accumulation; tolerance budget ~2e-2 vs bf16's ~5e-3).

Per-core dataflow, per 512-row x tile:
  - x^T via DMA xbar transpose (zero PE time, d = c*128+p chunk layout)
  - q^T = Wq^T @ x^T                                        (PE 16x512 rows)
  - per head: scores^T = kT_h^T @ qT_h   [77, 512]          (PE 512 rows)
              expT = exp(0.125*scores^T)                    (ACT)
              [attnU^T; den] = [v_h | 1]^T @ expT           (PE 512 rows)
                -> PSUM rows 0:64 unnormalized attn, rows 64:128 softmax
                   denominator broadcast 64x (ones-columns ride along free)
  - normalization on DVE in head pairs (same sub-row, adjacent chunks in one
    [128, 2x512] PSUM tile): reciprocal [64,1024] PSUM->SBUF + multiply
    -> attnT. (GPSIMD can't touch PSUM; TensorTensor allows only one PSUM
    operand, so recip+mul.)
  - out projection (PE 16x512) -> ACT paired copy PSUM->SBUF -> bias added
    in-place by the otherwise-idle GPSIMD -> DMA out.

Scheduling: the main loop is software-pipelined and interleaved at quad
granularity — iteration n emits x-load(n+2), q-projection(n+1), head quads
(n) and out-projection(n-1) alternately, so the in-order PE queue always
holds dense matmuls while a quad waits on ACT/DVE. Each quad's scores land
in the same [128, 2x512] PSUM tile that its attnU pass reclaims after the
(paired) exp reads it — one PSUM slot per quad end-to-end, which frees
enough banks for 3 rotation slots on the quad/out pool (6 banks) plus 2
single-bank q-projection slots.

Steady state per tile (2.4GHz PE): PE ~10.5us busy | ACT ~8.7 | DVE ~9.5 |
GPSIMD ~4.3 | DMA ~6.2; cost-model makespan 220.7us vs 310.2us baseline.

TRN2 allows 1 semaphore wait per instruction — generate_event_semaphores()
legalizes the multi-wait instructions Tile emits.
"""

import ml_dtypes
import numpy as np

import bass_rust as _bass_rust
import concourse.bass as bass
import concourse.mybir as mybir
import concourse.tile as tile
from concourse.bass_utils import run_bass_kernel_spmd
from concourse.masks import make_identity

N_CORES = 8
B, SQ, DM = 16, 4096, 512
SKV, DC = 77, 768
H, DH = 8, 64
INNER = 512
BPC = B // N_CORES  # batches per core

F32 = mybir.dt.float32
BF16 = mybir.dt.bfloat16

AF = mybir.ActivationFunctionType


def build_nc(trace_sim=False, variant="a", lag=1, wb=3, eb=3, ob=2):
    # variant "b" pairs the q-projection PSUM tiles (worse in sim: fewer
    # independent rotation slots); default "a" uses 4 single-bank q tiles
    q_pair = variant == "b"

    nc = bass.Bass()

    x_d = nc.dram_tensor("x", [BPC, SQ, DM], BF16, kind="ExternalInput")
    ctx_d = nc.dram_tensor("context", [BPC, SKV, DC], F32, kind="ExternalInput")
    wq_d = nc.dram_tensor("Wq", [DM, INNER], BF16, kind="ExternalInput")
    wk_d = nc.dram_tensor("Wk", [DC, INNER], BF16, kind="ExternalInput")
    wv_d = nc.dram_tensor("Wv", [DC, INNER], BF16, kind="ExternalInput")
    wo_d = nc.dram_tensor("Wout", [INNER, INNER], BF16, kind="ExternalInput")
    bo_d = nc.dram_tensor("bout", [INNER], F32, kind="ExternalInput")
    out_d = nc.dram_tensor("out", [BPC, SQ, DM], F32, kind="ExternalOutput")

    with tile.TileContext(nc, trace_sim=trace_sim) as tc:
        with (
            tc.tile_pool(name="const", bufs=1) as consts,
            tc.tile_pool(name="perbatch", bufs=2) as pb,
            tc.tile_pool(name="work", bufs=wb) as work,
            tc.tile_pool(name="exps", bufs=eb) as exps,
            tc.tile_pool(name="smalls", bufs=4) as smalls,
            tc.tile_pool(name="osbp", bufs=ob) as osbp,
            tc.tile_pool(name="pq", bufs=2, space="PSUM") as pq_p,
            tc.tile_pool(name="pu", bufs=3, space="PSUM") as pu_p,
        ):
            # ---- constants ----
            identity = consts.tile([128, 128], F32, tag="ident")
            make_identity(nc, identity)

            bias_bb = consts.tile([128, 2, INNER], F32, tag="bias")
            for t in range(2):
                nc.gpsimd.dma_start(
                    out=bias_bb[:, t, :], in_=bo_d[:].partition_broadcast(128)
                )

            # bf16 weights DMA'd directly; layout [p, c, e], row = c*128 + p —
            # matches both the DMA-xbar x^T layout and attnT's chunk layout.
            def load_w(dram, nchunk, tag):
                wt = consts.tile([128, nchunk, INNER], BF16, tag=tag)
                nc.sync.dma_start(
                    out=wt, in_=dram[:].rearrange("(c p) e -> p c e", p=128)
                )
                return wt

            def emit_xload(b, s0):
                x_bf = work.tile([128, 4, DM], BF16, tag="x")
                nc.sync.dma_start(
                    out=x_bf,
                    in_=x_d[b, s0:s0 + 512, :].rearrange(
                        "(t p) d -> p t d", p=128
                    ),
                )
                xT = work.tile([128, 4, 512], BF16, tag="xT")  # d = c*128+p
                for t in range(4):
                    nc.sync.dma_start_transpose(
                        out=xT[:, :, t * 128:(t + 1) * 128], in_=x_bf[:, t, :]
                    )
                return xT

            def emit_outpair(attnT, b, s0, tp):
                # one paired out tile; ACT copy-out; GPSIMD adds bias in SBUF
                po = pu_p.tile([128, 2, 512], F32, tag="u")
                for t01 in range(2):
                    t = 2 * tp + t01
                    for c in range(4):
                        nc.tensor.matmul(
                            out=po[:, t01, :],
                            lhsT=attnT[:, c, t * 128:(t + 1) * 128],
                            rhs=wo_sb[:, c, :],
                            start=(c == 0), stop=(c == 3),
                        )
                osb = osbp.tile([128, 2, 512], F32, tag="osb")
                nc.scalar.copy(out=osb, in_=po)
                nc.gpsimd.tensor_add(osb, osb, bias_bb)
                nc.sync.dma_start(
                    out=out_d[b, s0 + tp * 256:s0 + (tp + 1) * 256, :]
                    .rearrange("(t p) d -> p t d", p=128),
                    in_=osb,
                )

            def emit_kv(b, ctx_pre=None):
                # ---- context load + fp32 PE transpose (6x77 rows only) ----
                if ctx_pre is None:
                    ctx_sb = pb.tile([SKV, DC], F32, tag="ctx")
                    nc.sync.dma_start(out=ctx_sb, in_=ctx_d[b])
                else:
                    ctx_sb = ctx_pre

                ctxT = pb.tile([128, DC // 128, SKV], BF16, tag="ctxT")
                for jp in range(3):
                    pt = pu_p.tile([128, 2, 512], F32, tag="u", name=f"pt_{b}_{jp}")
                    for j01 in range(2):
                        j = 2 * jp + j01
                        nc.tensor.matmul(
                            out=pt[:, j01, 0:SKV],
                            lhsT=ctx_sb[:, j * 128:(j + 1) * 128],
                            rhs=identity[0:SKV, 0:SKV],
                            is_transpose=True, start=True, stop=True,
                        )
                    nc.scalar.copy(
                        out=ctxT[:, 2 * jp:2 * jp + 2, :], in_=pt[:, :, 0:SKV]
                    )

                # ---- kT = Wk^T @ ctx^T : [128e, 4, 77] ----
                kT = pb.tile([128, INNER // 128, SKV], BF16, tag="kT")
                for ip in range(2):
                    pk = pu_p.tile([128, 2, 512], F32, tag="u", name=f"pk_{b}_{ip}")
                    for i01 in range(2):
                        i = 2 * ip + i01
                        for j in range(DC // 128):
                            nc.tensor.matmul(
                                out=pk[:, i01, 0:SKV],
                                lhsT=wk_sb[:, j, i * 128:(i + 1) * 128],
                                rhs=ctxT[:, j, :],
                                start=(j == 0), stop=(j == DC // 128 - 1),
                            )
                    nc.scalar.copy(
                        out=kT[:, 2 * ip:2 * ip + 2, :], in_=pk[:, :, 0:SKV]
                    )

                # ---- v_aug[:, h, :] = [v_h | ones] : [77, 8, 128] ----
                v_aug = pb.tile([SKV, H, 128], BF16, tag="vaug")
                nc.vector.memset(v_aug[:, :, 64:128], 1.0)
                pv = pu_p.tile([128, 2, 512], F32, tag="u", name=f"pv_{b}")
                for j in range(DC // 128):
                    nc.tensor.matmul(
                        out=pv[0:SKV, 0, :],
                        lhsT=ctxT[:, j, :],
                        rhs=wv_sb[:, j, :],
                        start=(j == 0), stop=(j == DC // 128 - 1),
                    )
                nc.scalar.copy(
                    out=v_aug[:, :, 0:64],
                    in_=pv[0:SKV, 0, :].rearrange("k (h d) -> k h d", h=H),
                )
                return kT, v_aug

            def emit_qpair(cp, qT, xT):
                if q_pair:
                    pq = pq_p.tile([128, 2, 512], F32, tag="q")
                    for i01 in range(2):
                        i = 2 * cp + i01
                        for c in range(4):
                            nc.tensor.matmul(
                                out=pq[:, i01, :],
                                lhsT=wq_sb[:, c, i * 128:(i + 1) * 128],
                                rhs=xT[:, c, :],
                                start=(c == 0), stop=(c == 3),
                            )
                    nc.scalar.copy(out=qT[:, 2 * cp:2 * cp + 2, :], in_=pq)
                else:
                    for i01 in range(2):
                        i = 2 * cp + i01
                        pq = pq_p.tile([128, 512], F32, tag="q")
                        for c in range(4):
                            nc.tensor.matmul(
                                out=pq,
                                lhsT=wq_sb[:, c, i * 128:(i + 1) * 128],
                                rhs=xT[:, c, :],
                                start=(c == 0), stop=(c == 3),
                            )
                        nc.scalar.copy(out=qT[:, i, :], in_=pq)

            def emit_quad(cp, sub, qT, attnT, kT, v_aug):
                # scores land in the SAME pu tile that attnU reclaims after
                # the exp has read them — one PSUM slot per quad end-to-end
                r0 = sub * 64
                et = exps.tile([SKV, 2, 512], BF16, tag="expT")
                pa = pu_p.tile([128, 2, 512], F32, tag="u")
                for half in range(2):
                    i = 2 * cp + half
                    nc.tensor.matmul(
                        out=pa[0:SKV, half, :],
                        lhsT=kT[r0:r0 + 64, i, :],
                        rhs=qT[r0:r0 + 64, i, :],
                        start=True, stop=True,
                    )
                nc.scalar.activation(
                    out=et, in_=pa[0:SKV, :, :], func=AF.Exp, scale=0.125,
                )
                for half in range(2):
                    h = 2 * (2 * cp + half) + sub
                    nc.tensor.matmul(
                        out=pa[:, half, :],
                        lhsT=v_aug[:, h, :],
                        rhs=et[:, half, :],
                        start=True, stop=True,
                    )
                rr = smalls.tile([64, 2, 512], F32, tag="rr")
                nc.vector.reciprocal(out=rr, in_=pa[64:128, :, :])
                nc.vector.tensor_mul(
                    attnT[r0:r0 + 64, 2 * cp:2 * cp + 2, :],
                    pa[0:64, :, :],
                    rr,
                )

            # ---- startup: small ctx DMA first, wq, then tile-0/1 x so PE
            # can start early; remaining weights stream in behind ----
            tiles = [(b, st * 512) for b in range(BPC) for st in range(SQ // 512)]
            NT = len(tiles)

            ctx_sb0 = pb.tile([SKV, DC], F32, tag="ctx")
            nc.sync.dma_start(out=ctx_sb0, in_=ctx_d[0])
            wq_sb = load_w(wq_d, DM // 128, "wq")
            xTs = {0: emit_xload(*tiles[0]), 1: emit_xload(*tiles[1])}
            wk_sb = load_w(wk_d, DC // 128, "wk")
            wv_sb = load_w(wv_d, DC // 128, "wv")
            wo_sb = load_w(wo_d, INNER // 128, "wo")

            kvs = {0: emit_kv(0, ctx_pre=ctx_sb0)}
            qTs = {0: work.tile([128, 4, 512], BF16, tag="qT", name="qT_t0")}
            emit_qpair(0, qTs[0], xTs[0])
            emit_qpair(1, qTs[0], xTs[0])
            attns = {}

            # ---- software-pipelined main loop: iteration n interleaves the
            # latency-chained head quads of tile n (sparse PE) with the dense
            # q-projection of tile n+1 and out-projection of tile n-1, so the
            # in-order PE queue never drains while a quad waits on ACT/DVE ----
            for n in range(NT + lag):
                if n + 2 < NT:
                    xTs[n + 2] = emit_xload(*tiles[n + 2])
                    xTs.pop(n, None)
                if n + 1 < NT:
                    qTs[n + 1] = work.tile([128, 4, 512], BF16, tag="qT", name=f"qT_t{n + 1}")
                if n < NT:
                    attns[n] = work.tile([128, 4, 512], BF16, tag="attnT", name=f"attnT_t{n}")
                    kT, v_aug = kvs[tiles[n][0]]
                # next batch's k/v mid-batch so its PE work fills stalls
                # instead of serializing at the batch boundary
                if n < NT and tiles[n][1] == 5 * 512 and tiles[n][0] + 1 < BPC:
                    kvs[tiles[n][0] + 1] = emit_kv(tiles[n][0] + 1)

                if n + 1 < NT:
                    emit_qpair(0, qTs[n + 1], xTs[n + 1])
                if n < NT:
                    emit_quad(0, 0, qTs[n], attns[n], kT, v_aug)
                if n >= lag:
                    emit_outpair(attns[n - lag], *tiles[n - lag], 0)
                if n < NT:
                    emit_quad(0, 1, qTs[n], attns[n], kT, v_aug)
                if n + 1 < NT:
                    emit_qpair(1, qTs[n + 1], xTs[n + 1])
                if n < NT:
                    emit_quad(1, 0, qTs[n], attns[n], kT, v_aug)
                if n >= lag:
                    emit_outpair(attns[n - lag], *tiles[n - lag], 1)
                    attns.pop(n - lag)
                if n < NT:
                    emit_quad(1, 1, qTs[n], attns[n], kT, v_aug)
                qTs.pop(n - 1, None)

    # TRN2 hardware allows at most 1 semaphore wait per instruction; split
    # multi-wait instructions into standalone EventSemaphore waits.
    _bass_rust.generate_event_semaphores(nc)
    return nc


_NC_CACHE = None


def kernel(x, context, Wq, Wk, Wv, Wout, bout):
    global _NC_CACHE
    if _NC_CACHE is None:
        _NC_CACHE = build_nc()
    nc = _NC_CACHE

    f32 = lambda a: np.ascontiguousarray(np.asarray(a), dtype=np.float32)
    bf = lambda a: np.ascontiguousarray(
        np.asarray(np.asarray(a, dtype=np.float32), dtype=ml_dtypes.bfloat16)
    )
    x, context, bout = bf(x), f32(context), f32(bout)
    Wq, Wk, Wv, Wout = bf(Wq), bf(Wk), bf(Wv), bf(Wout)

    in_maps = [
        {
            "x": x[c * BPC:(c + 1) * BPC],
            "context": context[c * BPC:(c + 1) * BPC],
            "Wq": Wq, "Wk": Wk, "Wv": Wv, "Wout": Wout, "bout": bout,
        }
        for c in range(N_CORES)
    ]
    res = run_bass_kernel_spmd(nc, in_maps, core_ids=list(range(N_CORES)))
    return np.concatenate([r["out"] for r in res.results], axis=0)
